# revision 23
# baseline (speedup 1.0000x reference)
"""CapsNet routing layer (nn_CapsLayer) on 8 Trainium2 NeuronCores.

reference:
    u_hat = einsum("ncoi,bci->bnco", W[0], x)         # B,N,C,O = 1024,2,512,64
    3 dynamic-routing iterations (softmax over n, weighted sum over c,
    squash, agreement update); returns v from iteration 3.

Strategy (in-caps sharded, hardcoded shapes):
  - 8 cores x 64 in-caps, every core sees the full batch. Per-core DMA
    is 64 MiB of x + 8 MiB of W (resident in SBUF) instead of the
    128+ MiB a batch shard would need (W replicated).
  - GEMM is a bf16 hi/lo x3 decomposition (xh*Wh + xh*Wl + xl*Wh, fp32
    PSUM accumulate): ~1e-5 relative accuracy at bf16 PE rate with
    fast-weight-load (128 stationary columns = one 128-sample chunk).
  - The batch is processed as 8 pipelined chunks of 128 samples; each
    chunk's u_hat tile (p=b128, f=(n,c,o), 4 MiB fp32) stays in SBUF for
    all 3 routing iterations. Chunk k+1's GEMM overlaps chunk k's
    routing.
  - The weighted sum over in-caps needs a cross-core reduction: one
    64 KB AllReduce per (chunk, iteration) = 24 total, overlapped with
    other chunks' compute. Every core then computes identical v
    (redundantly) and writes the full output; the host takes core 0's.
  - softmax over N=2 collapses to sigmoid(b0-b1); iteration 1 uses the
    exact coefficients 0.5; iteration 3 skips the dead y/b update.
    Routing is single-instruction TT/TR passes on DVE (GPSIMD offload
    measured net-negative: SBUF port contention stretches DVE slices).
"""
import os
import sys
import types

sys.path.insert(0, "/opt/trn_rl_repo")

import numpy as np
import ml_dtypes
import concourse.bass as bass
import concourse.mybir as mybir
import concourse.tile as tile
from concourse.bass_utils import run_bass_kernel_spmd

BF16NP = ml_dtypes.bfloat16

B, NCAPS, C, ICH, OCH = 1024, 2, 512, 256, 64
ITERATIONS = 3
NCORES = 8
CPC = C // NCORES            # in-caps per core = 64
NBCH = 8                     # batch chunks
BCH = B // NBCH              # samples per chunk = 128
KH = 2                       # K halves (ICH = 2*128)
CG = 8                       # c's per GEMM/DMA group

FP32 = mybir.dt.float32
BF16 = mybir.dt.bfloat16
ADD = mybir.AluOpType.add
MULT = mybir.AluOpType.mult
SUB = mybir.AluOpType.subtract
AF = mybir.ActivationFunctionType
AX = mybir.AxisListType

LAST_EXEC_NS = None


def _install_profile_hook():
    """antenv.axon_hooks is absent in this image; recreate it so
    run_bass_kernel_spmd(trace=True)/BASS_TRACE can report exec_time_ns."""
    if "antenv.axon_hooks" in sys.modules:
        return
    mod = types.ModuleType("antenv.axon_hooks")
    mod._hook = None
    mod.set_axon_ntff_profile_hook = lambda h: setattr(mod, "_hook", h)
    mod.get_axon_ntff_profile_hook = lambda: mod._hook
    sys.modules["antenv.axon_hooks"] = mod
    try:
        from trn_agent_boot.trn_boot import _ntff_profile_via_ctypes

        hook = _ntff_profile_via_ctypes("/opt/axon/libaxon_pjrt.so")
        if hook is not None:
            mod._hook = hook
    except Exception:
        pass


def _split_sync_waits(nc, max_waits=1):
    """walrus setupSyncWait rejects instructions with more than one sem
    wait; hoist extras onto same-engine InstNoOp's placed just before."""
    for f in nc.m.functions:
        for bb in f.blocks:
            out = []
            changed = False
            for inst in bb.instructions:
                si = inst.sync_info
                waits = list(si.on_wait) if si is not None and si.on_wait else []
                if len(waits) > max_waits:
                    extra, keep = waits[:-max_waits], waits[-max_waits:]
                    for g, w in enumerate(extra):
                        out.append(
                            mybir.InstNoOp(
                                name=f"{inst.name}_wsplit{g}",
                                engine=inst.engine,
                                bass_nofuse=True,
                                sync_info=mybir.SyncInfo(on_wait=[w], on_update=[]),
                            )
                        )
                    inst.sync_info = mybir.SyncInfo(
                        on_wait=keep,
                        on_update=list(si.on_update) if si.on_update else [],
                    )
                    changed = True
                out.append(inst)
            if changed:
                bb.instructions = out


def build_kernel(split_waits=True):
    nc = bass.Bass(
        "TRN2", target_bir_lowering=False, debug=False, num_devices=NCORES
    )
    # x shard: [h, i, bchunk, c, b] bf16 (hi and lo)
    xth = nc.dram_tensor("xth", [KH, 128, NBCH, CPC, BCH], BF16, kind="ExternalInput").ap()
    xtl = nc.dram_tensor("xtl", [KH, 128, NBCH, CPC, BCH], BF16, kind="ExternalInput").ap()
    # W shard: [h, i, c, (n,o)] bf16 (hi and lo)
    wth = nc.dram_tensor("wth", [KH, 128, CPC, NCAPS * OCH], BF16, kind="ExternalInput").ap()
    wtl = nc.dram_tensor("wtl", [KH, 128, CPC, NCAPS * OCH], BF16, kind="ExternalInput").ap()
    out = nc.dram_tensor("out", [B, NCAPS, OCH], FP32, kind="ExternalOutput").ap()

    with tile.TileContext(nc) as tc:
        with (
            tc.tile_pool(name="xin", bufs=3) as xpool,
            tc.tile_pool(name="psum", bufs=3, space="PSUM") as pspool,
            tc.tile_pool(name="ubuf", bufs=2) as upool,
            tc.tile_pool(name="dram", bufs=4, space="DRAM") as drpool,
        ):
            # resident W: [h][hi/lo] tiles (128i, c*no)
            wsb = {}
            for h in range(KH):
                for hl, src in (("h", wth), ("l", wtl)):
                    t = nc.alloc_sbuf_tensor(f"w{hl}{h}", [128, CPC * NCAPS * OCH], BF16).ap()
                    nc.sync.dma_start(t[:], src[h].rearrange("i c f -> i (c f)"))
                    wsb[(h, hl)] = t

            # routing tensors, duplicated by chunk parity so even/odd
            # chunks' routing can interleave (hides AllReduce stalls)
            d_all = nc.alloc_sbuf_tensor("d_all", [128, NBCH, CPC], FP32).ap()
            P = 2
            wn = [nc.alloc_sbuf_tensor(f"wscr{p}", [128, CPC, OCH], FP32).ap() for p in range(P)]
            coeff = [nc.alloc_sbuf_tensor(f"coeff{p}", [128, NCAPS, CPC], FP32).ap() for p in range(P)]
            sp = [nc.alloc_sbuf_tensor(f"sp{p}", [128, NCAPS, OCH], FP32).ap() for p in range(P)]
            ss = [nc.alloc_sbuf_tensor(f"ss{p}", [128, NCAPS, OCH], FP32).ap() for p in range(P)]
            s2 = [nc.alloc_sbuf_tensor(f"s2{p}", [128, NCAPS, OCH], FP32).ap() for p in range(P)]
            sq = [nc.alloc_sbuf_tensor(f"sq{p}", [128, NCAPS], FP32).ap() for p in range(P)]
            rr = [nc.alloc_sbuf_tensor(f"rr{p}", [128, NCAPS], FP32).ap() for p in range(P)]
            den = [nc.alloc_sbuf_tensor(f"den{p}", [128, NCAPS], FP32).ap() for p in range(P)]
            rec = [nc.alloc_sbuf_tensor(f"rec{p}", [128, NCAPS], FP32).ap() for p in range(P)]
            mmv = [nc.alloc_sbuf_tensor(f"mmv{p}", [128, NCAPS], FP32).ap() for p in range(P)]
            v = [nc.alloc_sbuf_tensor(f"v{p}", [128, NCAPS, OCH], FP32).ap() for p in range(P)]
            y = [nc.alloc_sbuf_tensor(f"y{p}", [128, NCAPS, CPC], FP32).ap() for p in range(P)]
            dd = [nc.alloc_sbuf_tensor(f"dd{p}", [128, CPC], FP32).ap() for p in range(P)]

            for bk in range(NBCH):
                u = upool.tile([128, NCAPS, CPC, OCH], FP32, tag="u")

                # ---- GEMM for this chunk ----
                for cg in range(CPC // CG):
                    c0 = cg * CG
                    xt = {}
                    for h in range(KH):
                        for hl, src in (("h", xth), ("l", xtl)):
                            t = xpool.tile([128, CG, BCH], BF16, tag=f"x{hl}{h}")
                            nc.sync.dma_start(t[:], src[h, :, bk, c0 : c0 + CG, :])
                            xt[(h, hl)] = t
                    pg = pspool.tile([BCH, CG, NCAPS * OCH], FP32, tag="pg")
                    for j in range(CG):
                        c = c0 + j
                        terms = []
                        for h in range(KH):
                            wslice_h = wsb[(h, "h")][
                                :, c * NCAPS * OCH : (c + 1) * NCAPS * OCH
                            ]
                            wslice_l = wsb[(h, "l")][
                                :, c * NCAPS * OCH : (c + 1) * NCAPS * OCH
                            ]
                            terms.append((xt[(h, "h")][:, j, :], wslice_h))
                            terms.append((xt[(h, "h")][:, j, :], wslice_l))
                            terms.append((xt[(h, "l")][:, j, :], wslice_h))
                        for ti, (sta, mov) in enumerate(terms):
                            nc.tensor.matmul(
                                pg[:, j, :],
                                lhsT=sta,
                                rhs=mov,
                                start=(ti == 0),
                                stop=(ti == len(terms) - 1),
                            )
                    # PSUM (b, (c,n,o)) -> u (b, (n,c,o)): strided ACT copy
                    nc.scalar.copy(
                        u[:, :, c0 : c0 + CG, :],
                        pg[:].rearrange("b c (n o) -> b n c o", n=NCAPS),
                    )

                # ---- routing for this chunk ----
                p = bk % 2
                d = d_all[:, bk, :]
                for it in range(ITERATIONS):
                    # s_partial = sum_c coeff * u (coeff = 0.5 exactly on it 0)
                    for n in range(NCAPS):
                        if it == 0:
                            nc.vector.tensor_reduce(
                                sp[p][:, n, :].unsqueeze(2),
                                u[:, n].transpose([0, 2, 1]),
                                axis=AX.X,
                                op=ADD,
                            )
                        else:
                            cb = (
                                coeff[p][:, n, :]
                                .unsqueeze(2)
                                .broadcast_to((128, CPC, OCH))
                            )
                            nc.vector.tensor_tensor(wn[p][:], u[:, n], cb, op=MULT)
                            nc.vector.tensor_reduce(
                                sp[p][:, n, :].unsqueeze(2),
                                wn[p][:].transpose([0, 2, 1]),
                                axis=AX.X,
                                op=ADD,
                            )
                    if it == 0:
                        nc.vector.tensor_scalar(sp[p][:], sp[p][:], 0.5, None, op0=MULT)
                    # AllReduce s_partial across the 8 c-shards
                    bi = drpool.tile([128, NCAPS * OCH], FP32, tag="bi")
                    bo = drpool.tile([128, NCAPS * OCH], FP32, tag="bo")
                    nc.sync.dma_start(bi[:], sp[p][:].rearrange("p n o -> p (n o)"))
                    nc.gpsimd.collective_compute(
                        "AllReduce",
                        ADD,
                        replica_groups=[list(range(NCORES))],
                        ins=[bi[:].opt()],
                        outs=[bo[:].opt()],
                    )
                    nc.sync.dma_start(ss[p][:].rearrange("p n o -> p (n o)"), bo[:])

                    # squash: v = ss * sq / ((1+sq) sqrt(sq))
                    nc.vector.tensor_tensor(s2[p][:], ss[p][:], ss[p][:], op=MULT)
                    nc.vector.tensor_reduce(
                        sq[p][:].unsqueeze(2), s2[p][:], axis=AX.X, op=ADD
                    )
                    nc.scalar.activation(rr[p][:], sq[p][:], AF.Sqrt)
                    nc.vector.tensor_scalar(den[p][:], sq[p][:], 1.0, None, op0=ADD)
                    nc.vector.tensor_tensor(den[p][:], den[p][:], rr[p][:], op=MULT)
                    nc.vector.reciprocal(rec[p][:], den[p][:])
                    nc.vector.tensor_tensor(mmv[p][:], sq[p][:], rec[p][:], op=MULT)
                    vb2 = mmv[p][:].unsqueeze(2).broadcast_to((128, NCAPS, OCH))
                    nc.vector.tensor_tensor(v[p][:], ss[p][:], vb2, op=MULT)

                    if it == ITERATIONS - 1:
                        break

                    # y = sum_o u * v
                    for n in range(NCAPS):
                        vb = (
                            v[p][:, n, :]
                            .unsqueeze(1)
                            .broadcast_to((128, CPC, OCH))
                        )
                        nc.vector.tensor_tensor(wn[p][:], u[:, n], vb, op=MULT)
                        nc.vector.tensor_reduce(
                            y[p][:, n, :].unsqueeze(2), wn[p][:], axis=AX.X, op=ADD
                        )
                    # d += y0 - y1 ; coeff = sigmoid(+/-d)
                    nc.vector.tensor_tensor(dd[p][:], y[p][:, 0, :], y[p][:, 1, :], op=SUB)
                    if it == 0:
                        nc.vector.tensor_copy(d, dd[p][:])
                    else:
                        nc.vector.tensor_tensor(d, d, dd[p][:], op=ADD)
                    nc.scalar.activation(coeff[p][:, 0, :], d, AF.Sigmoid)
                    nc.scalar.activation(coeff[p][:, 1, :], d, AF.Sigmoid, scale=-1.0)

                # every core writes the (identical) chunk result
                nc.sync.dma_start(out[bk * BCH : (bk + 1) * BCH, :, :], v[p][:])

    if split_waits:
        _split_sync_waits(nc)
    return nc


def _prep_inputs(x, W):
    x = np.ascontiguousarray(x, dtype=np.float32)
    W0 = np.ascontiguousarray(W.reshape(NCAPS, C, OCH, ICH), dtype=np.float32)
    xth_cores, xtl_cores, wth_cores, wtl_cores = [], [], [], []
    for k in range(NCORES):
        cs = k * CPC
        xc = x[:, cs : cs + CPC, :]  # (B, 64, 256)
        x6 = xc.reshape(NBCH, BCH, CPC, KH, 128)
        xt = np.ascontiguousarray(x6.transpose(3, 4, 0, 2, 1))  # (h,i,bk,c,b)
        xh = xt.astype(BF16NP)
        xlo = (xt - xh.astype(np.float32)).astype(BF16NP)
        xth_cores.append(xh)
        xtl_cores.append(xlo)
        Wc = W0[:, cs : cs + CPC]  # (2, 64, 64, 256)
        w5 = Wc.reshape(NCAPS, CPC, OCH, KH, 128)
        wt = np.ascontiguousarray(w5.transpose(3, 4, 1, 0, 2)).reshape(
            KH, 128, CPC, NCAPS * OCH
        )
        wh = wt.astype(BF16NP)
        wlo = (wt - wh.astype(np.float32)).astype(BF16NP)
        wth_cores.append(wh)
        wtl_cores.append(wlo)
    return xth_cores, xtl_cores, wth_cores, wtl_cores


_NC_CACHE = {}


def kernel(x, W):
    global LAST_EXEC_NS
    _install_profile_hook()
    if "nc" not in _NC_CACHE:
        _NC_CACHE["nc"] = build_kernel()
    nc = _NC_CACHE["nc"]
    xth, xtl, wth, wtl = _prep_inputs(np.asarray(x), np.asarray(W))
    in_maps = [
        {"xth": xth[k], "xtl": xtl[k], "wth": wth[k], "wtl": wtl[k]}
        for k in range(NCORES)
    ]
    trace = bool(os.environ.get("CAPS_TRACE"))
    res = run_bass_kernel_spmd(nc, in_maps, list(range(NCORES)), trace=trace)
    LAST_EXEC_NS = res.exec_time_ns
    return res.results[0]["out"].astype(np.float32)


# revision 24
# speedup vs baseline: 1.2438x; 1.2438x over previous
"""CapsNet routing layer (nn_CapsLayer) on 8 Trainium2 NeuronCores.

reference:
    u_hat = einsum("ncoi,bci->bnco", W[0], x)         # B,N,C,O = 1024,2,512,64
    3 dynamic-routing iterations (softmax over n, weighted sum over c,
    squash, agreement update); returns v from iteration 3.

Strategy (in-caps sharded, hardcoded shapes):
  - 8 cores x 64 in-caps, every core sees the full batch. Per-core DMA
    is 64 MiB of x + 8 MiB of W (resident in SBUF) instead of the
    128+ MiB a batch shard would need (W replicated).
  - GEMM is a bf16 hi/lo x3 decomposition (xh*Wh + xh*Wl + xl*Wh, fp32
    PSUM accumulate): ~1e-5 relative accuracy at bf16 PE rate with
    fast-weight-load (128 stationary columns = one 128-sample chunk).
  - The batch is processed as 8 pipelined chunks of 128 samples; each
    chunk's u_hat tile (p=b128, f=(n,c,o), 4 MiB fp32) stays in SBUF for
    all 3 routing iterations. Chunk k+1's GEMM overlaps chunk k's
    routing.
  - The weighted sum over in-caps needs a cross-core reduction: one
    64 KB AllReduce per (chunk, iteration) = 24 total, overlapped with
    other chunks' compute. Every core then computes identical v
    (redundantly) and writes the full output; the host takes core 0's.
  - softmax over N=2 collapses to sigmoid(b0-b1); iteration 1 uses the
    exact coefficients 0.5; iteration 3 skips the dead y/b update.
    Routing is single-instruction TT/TR passes on DVE (GPSIMD offload
    measured net-negative: SBUF port contention stretches DVE slices).
"""
import os
import sys
import types

sys.path.insert(0, "/opt/trn_rl_repo")

import numpy as np
import ml_dtypes
import concourse.bass as bass
import concourse.mybir as mybir
import concourse.tile as tile
from concourse.bass_utils import run_bass_kernel_spmd

BF16NP = ml_dtypes.bfloat16

B, NCAPS, C, ICH, OCH = 1024, 2, 512, 256, 64
ITERATIONS = 3
NCORES = 8
CPC = C // NCORES            # in-caps per core = 64
NBCH = 8                     # batch chunks
BCH = B // NBCH              # samples per chunk = 128
KH = 2                       # K halves (ICH = 2*128)
CG = 8                       # c's per GEMM/DMA group

FP32 = mybir.dt.float32
BF16 = mybir.dt.bfloat16
ADD = mybir.AluOpType.add
MULT = mybir.AluOpType.mult
SUB = mybir.AluOpType.subtract
AF = mybir.ActivationFunctionType
AX = mybir.AxisListType

LAST_EXEC_NS = None


def _install_profile_hook():
    """antenv.axon_hooks is absent in this image; recreate it so
    run_bass_kernel_spmd(trace=True)/BASS_TRACE can report exec_time_ns."""
    if "antenv.axon_hooks" in sys.modules:
        return
    mod = types.ModuleType("antenv.axon_hooks")
    mod._hook = None
    mod.set_axon_ntff_profile_hook = lambda h: setattr(mod, "_hook", h)
    mod.get_axon_ntff_profile_hook = lambda: mod._hook
    sys.modules["antenv.axon_hooks"] = mod
    try:
        from trn_agent_boot.trn_boot import _ntff_profile_via_ctypes

        hook = _ntff_profile_via_ctypes("/opt/axon/libaxon_pjrt.so")
        if hook is not None:
            mod._hook = hook
    except Exception:
        pass


def _split_sync_waits(nc, max_waits=1):
    """walrus setupSyncWait rejects instructions with more than one sem
    wait; hoist extras onto same-engine InstNoOp's placed just before."""
    for f in nc.m.functions:
        for bb in f.blocks:
            out = []
            changed = False
            for inst in bb.instructions:
                si = inst.sync_info
                waits = list(si.on_wait) if si is not None and si.on_wait else []
                if len(waits) > max_waits:
                    extra, keep = waits[:-max_waits], waits[-max_waits:]
                    for g, w in enumerate(extra):
                        out.append(
                            mybir.InstNoOp(
                                name=f"{inst.name}_wsplit{g}",
                                engine=inst.engine,
                                bass_nofuse=True,
                                sync_info=mybir.SyncInfo(on_wait=[w], on_update=[]),
                            )
                        )
                    inst.sync_info = mybir.SyncInfo(
                        on_wait=keep,
                        on_update=list(si.on_update) if si.on_update else [],
                    )
                    changed = True
                out.append(inst)
            if changed:
                bb.instructions = out


def build_kernel(split_waits=True):
    nc = bass.Bass(
        "TRN2", target_bir_lowering=False, debug=False, num_devices=NCORES
    )
    # x shard: [h, i, bchunk, c, b] bf16 (hi and lo)
    xth = nc.dram_tensor("xth", [KH, 128, NBCH, CPC, BCH], BF16, kind="ExternalInput").ap()
    xtl = nc.dram_tensor("xtl", [KH, 128, NBCH, CPC, BCH], BF16, kind="ExternalInput").ap()
    # W shard: [h, i, c, (n,o)] bf16 (hi and lo)
    wth = nc.dram_tensor("wth", [KH, 128, CPC, NCAPS * OCH], BF16, kind="ExternalInput").ap()
    wtl = nc.dram_tensor("wtl", [KH, 128, CPC, NCAPS * OCH], BF16, kind="ExternalInput").ap()
    out = nc.dram_tensor("out", [B, NCAPS, OCH], FP32, kind="ExternalOutput").ap()

    with tile.TileContext(nc) as tc:
        with (
            tc.tile_pool(name="xin", bufs=3) as xpool,
            tc.tile_pool(name="psum", bufs=3, space="PSUM") as pspool,
            tc.tile_pool(name="ubuf", bufs=2) as upool,
            tc.tile_pool(name="dram", bufs=4, space="DRAM") as drpool,
        ):
            # resident W: [h][hi/lo] tiles (128i, c*no)
            wsb = {}
            for h in range(KH):
                for hl, src in (("h", wth), ("l", wtl)):
                    t = nc.alloc_sbuf_tensor(f"w{hl}{h}", [128, CPC * NCAPS * OCH], BF16).ap()
                    nc.sync.dma_start(t[:], src[h].rearrange("i c f -> i (c f)"))
                    wsb[(h, hl)] = t

            # routing tensors, duplicated by chunk parity so even/odd
            # chunks' routing can interleave (hides AllReduce stalls)
            d_all = nc.alloc_sbuf_tensor("d_all", [128, NBCH, CPC], FP32).ap()
            P = 2
            wn = [nc.alloc_sbuf_tensor(f"wscr{p}", [128, CPC, OCH], FP32).ap() for p in range(P)]
            coeff = [nc.alloc_sbuf_tensor(f"coeff{p}", [128, NCAPS, CPC], FP32).ap() for p in range(P)]
            sp = [nc.alloc_sbuf_tensor(f"sp{p}", [128, NCAPS, OCH], FP32).ap() for p in range(P)]
            ss = [nc.alloc_sbuf_tensor(f"ss{p}", [128, NCAPS, OCH], FP32).ap() for p in range(P)]
            s2 = [nc.alloc_sbuf_tensor(f"s2{p}", [128, NCAPS, OCH], FP32).ap() for p in range(P)]
            sq = [nc.alloc_sbuf_tensor(f"sq{p}", [128, NCAPS], FP32).ap() for p in range(P)]
            rr = [nc.alloc_sbuf_tensor(f"rr{p}", [128, NCAPS], FP32).ap() for p in range(P)]
            den = [nc.alloc_sbuf_tensor(f"den{p}", [128, NCAPS], FP32).ap() for p in range(P)]
            rec = [nc.alloc_sbuf_tensor(f"rec{p}", [128, NCAPS], FP32).ap() for p in range(P)]
            mmv = [nc.alloc_sbuf_tensor(f"mmv{p}", [128, NCAPS], FP32).ap() for p in range(P)]
            v = [nc.alloc_sbuf_tensor(f"v{p}", [128, NCAPS, OCH], FP32).ap() for p in range(P)]
            y = [nc.alloc_sbuf_tensor(f"y{p}", [128, NCAPS, CPC], FP32).ap() for p in range(P)]
            dd = [nc.alloc_sbuf_tensor(f"dd{p}", [128, CPC], FP32).ap() for p in range(P)]

            def gemm_chunk(bk):
                u = upool.tile([128, NCAPS, CPC, OCH], FP32, tag="u")
                # ---- GEMM for this chunk ----
                for cg in range(CPC // CG):
                    c0 = cg * CG
                    xt = {}
                    for h in range(KH):
                        for hl, src in (("h", xth), ("l", xtl)):
                            t = xpool.tile([128, CG, BCH], BF16, tag=f"x{hl}{h}")
                            nc.sync.dma_start(t[:], src[h, :, bk, c0 : c0 + CG, :])
                            xt[(h, hl)] = t
                    pg = pspool.tile([BCH, CG, NCAPS * OCH], FP32, tag="pg")
                    for j in range(CG):
                        c = c0 + j
                        terms = []
                        for h in range(KH):
                            wslice_h = wsb[(h, "h")][
                                :, c * NCAPS * OCH : (c + 1) * NCAPS * OCH
                            ]
                            wslice_l = wsb[(h, "l")][
                                :, c * NCAPS * OCH : (c + 1) * NCAPS * OCH
                            ]
                            terms.append((xt[(h, "h")][:, j, :], wslice_h))
                            terms.append((xt[(h, "h")][:, j, :], wslice_l))
                            terms.append((xt[(h, "l")][:, j, :], wslice_h))
                        for ti, (sta, mov) in enumerate(terms):
                            nc.tensor.matmul(
                                pg[:, j, :],
                                lhsT=sta,
                                rhs=mov,
                                start=(ti == 0),
                                stop=(ti == len(terms) - 1),
                            )
                    # PSUM (b, (c,n,o)) -> u (b, (n,c,o)): strided ACT copy
                    nc.scalar.copy(
                        u[:, :, c0 : c0 + CG, :],
                        pg[:].rearrange("b c (n o) -> b n c o", n=NCAPS),
                    )
                return u

            def route_iter(bk, u, it):
                p = bk % 2
                d = d_all[:, bk, :]
                if True:
                    # s_partial = sum_c coeff * u (coeff = 0.5 exactly on it 0)
                    for n in range(NCAPS):
                        if it == 0:
                            nc.vector.tensor_reduce(
                                sp[p][:, n, :].unsqueeze(2),
                                u[:, n].transpose([0, 2, 1]),
                                axis=AX.X,
                                op=ADD,
                            )
                        else:
                            cb = (
                                coeff[p][:, n, :]
                                .unsqueeze(2)
                                .broadcast_to((128, CPC, OCH))
                            )
                            nc.vector.tensor_tensor(wn[p][:], u[:, n], cb, op=MULT)
                            nc.vector.tensor_reduce(
                                sp[p][:, n, :].unsqueeze(2),
                                wn[p][:].transpose([0, 2, 1]),
                                axis=AX.X,
                                op=ADD,
                            )
                    if it == 0:
                        nc.vector.tensor_scalar(sp[p][:], sp[p][:], 0.5, None, op0=MULT)
                    # AllReduce s_partial across the 8 c-shards
                    bi = drpool.tile([128, NCAPS * OCH], FP32, tag="bi")
                    bo = drpool.tile([128, NCAPS * OCH], FP32, tag="bo")
                    nc.sync.dma_start(bi[:], sp[p][:].rearrange("p n o -> p (n o)"))
                    nc.gpsimd.collective_compute(
                        "AllReduce",
                        ADD,
                        replica_groups=[list(range(NCORES))],
                        ins=[bi[:].opt()],
                        outs=[bo[:].opt()],
                    )
                    nc.sync.dma_start(ss[p][:].rearrange("p n o -> p (n o)"), bo[:])

                    # squash: v = ss * sq / ((1+sq) sqrt(sq))
                    nc.vector.tensor_tensor(s2[p][:], ss[p][:], ss[p][:], op=MULT)
                    nc.vector.tensor_reduce(
                        sq[p][:].unsqueeze(2), s2[p][:], axis=AX.X, op=ADD
                    )
                    nc.scalar.activation(rr[p][:], sq[p][:], AF.Sqrt)
                    nc.vector.tensor_scalar(den[p][:], sq[p][:], 1.0, None, op0=ADD)
                    nc.vector.tensor_tensor(den[p][:], den[p][:], rr[p][:], op=MULT)
                    nc.vector.reciprocal(rec[p][:], den[p][:])
                    nc.vector.tensor_tensor(mmv[p][:], sq[p][:], rec[p][:], op=MULT)
                    vb2 = mmv[p][:].unsqueeze(2).broadcast_to((128, NCAPS, OCH))
                    nc.vector.tensor_tensor(v[p][:], ss[p][:], vb2, op=MULT)

                    if it == ITERATIONS - 1:
                        return

                    # y = sum_o u * v
                    for n in range(NCAPS):
                        vb = (
                            v[p][:, n, :]
                            .unsqueeze(1)
                            .broadcast_to((128, CPC, OCH))
                        )
                        nc.vector.tensor_tensor(wn[p][:], u[:, n], vb, op=MULT)
                        nc.vector.tensor_reduce(
                            y[p][:, n, :].unsqueeze(2), wn[p][:], axis=AX.X, op=ADD
                        )
                    # d += y0 - y1 ; coeff = sigmoid(+/-d)
                    nc.vector.tensor_tensor(dd[p][:], y[p][:, 0, :], y[p][:, 1, :], op=SUB)
                    if it == 0:
                        nc.vector.tensor_copy(d, dd[p][:])
                    else:
                        nc.vector.tensor_tensor(d, d, dd[p][:], op=ADD)
                    nc.scalar.activation(coeff[p][:, 0, :], d, AF.Sigmoid)
                    nc.scalar.activation(coeff[p][:, 1, :], d, AF.Sigmoid, scale=-1.0)

            # chunk pairs: interleave iteration emission so one chunk's
            # AllReduce stall hides behind the partner's DVE work
            for pr in range(NBCH // 2):
                bks = (2 * pr, 2 * pr + 1)
                us = {bk: gemm_chunk(bk) for bk in bks}
                for it in range(ITERATIONS):
                    for bk in bks:
                        route_iter(bk, us[bk], it)
                for bk in bks:
                    # every core writes the (identical) chunk result
                    nc.sync.dma_start(
                        out[bk * BCH : (bk + 1) * BCH, :, :], v[bk % 2][:]
                    )

    if split_waits:
        _split_sync_waits(nc)
    return nc


def _prep_inputs(x, W):
    x = np.ascontiguousarray(x, dtype=np.float32)
    W0 = np.ascontiguousarray(W.reshape(NCAPS, C, OCH, ICH), dtype=np.float32)
    xth_cores, xtl_cores, wth_cores, wtl_cores = [], [], [], []
    for k in range(NCORES):
        cs = k * CPC
        xc = x[:, cs : cs + CPC, :]  # (B, 64, 256)
        x6 = xc.reshape(NBCH, BCH, CPC, KH, 128)
        xt = np.ascontiguousarray(x6.transpose(3, 4, 0, 2, 1))  # (h,i,bk,c,b)
        xh = xt.astype(BF16NP)
        xlo = (xt - xh.astype(np.float32)).astype(BF16NP)
        xth_cores.append(xh)
        xtl_cores.append(xlo)
        Wc = W0[:, cs : cs + CPC]  # (2, 64, 64, 256)
        w5 = Wc.reshape(NCAPS, CPC, OCH, KH, 128)
        wt = np.ascontiguousarray(w5.transpose(3, 4, 1, 0, 2)).reshape(
            KH, 128, CPC, NCAPS * OCH
        )
        wh = wt.astype(BF16NP)
        wlo = (wt - wh.astype(np.float32)).astype(BF16NP)
        wth_cores.append(wh)
        wtl_cores.append(wlo)
    return xth_cores, xtl_cores, wth_cores, wtl_cores


_NC_CACHE = {}


def kernel(x, W):
    global LAST_EXEC_NS
    _install_profile_hook()
    if "nc" not in _NC_CACHE:
        _NC_CACHE["nc"] = build_kernel()
    nc = _NC_CACHE["nc"]
    xth, xtl, wth, wtl = _prep_inputs(np.asarray(x), np.asarray(W))
    in_maps = [
        {"xth": xth[k], "xtl": xtl[k], "wth": wth[k], "wtl": wtl[k]}
        for k in range(NCORES)
    ]
    trace = bool(os.environ.get("CAPS_TRACE"))
    res = run_bass_kernel_spmd(nc, in_maps, list(range(NCORES)), trace=trace)
    LAST_EXEC_NS = res.exec_time_ns
    return res.results[0]["out"].astype(np.float32)
